# revision 42
# baseline (speedup 1.0000x reference)
"""GPT forward pass on 8 Trainium2 NeuronCores.

Sharding: token-parallel trunk (core c owns rows 128c..128c+127 of each of
the 4 sequences = 512 tokens), head-parallel attention (core c owns heads
2c, 2c+1 over ALL tokens -- causal triangular work is then uniform across
cores, so SPMD loop bounds can skip the upper triangle), vocab-sharded
lm_head (4000 cols/core).

Per layer: LN1 -> AllGather h^T (fp8, 0.5MB) -> QKV for my 2 heads over all
4096 tokens (K/V stay SBUF-resident, no DRAM KV) -> causal scores/softmax/PV
-> AllToAll O back to token shards (fp8, 0.5MB) -> Wo + residual -> LN2 ->
FFN -> residual.

All big GEMMs run fp8(e4m3) DoubleRow (K=256/instruction, 2x PE rate), with
fp32 PSUM. Attention matmuls (scores, PV) are bf16. LayerNorm gains/biases
are folded into the following weights on the host; weights carry power-of-2
scales (8x/64x) to keep fp8 out of the subnormal range, descaled via free
scale slots (exp scale, gelu scale, copy scale). Residual stream is fp32.
Logits leave the device as bf16 at 64x scale; the host descales and adds
blm (+ lnf_b @ Wlm).
"""

import os
import sys

for _p in ("/opt/trn_rl_repo",):
    if os.path.isdir(_p) and _p not in sys.path:
        sys.path.insert(0, _p)

import numpy as np
import ml_dtypes

BF16NP = ml_dtypes.bfloat16
F8NP = ml_dtypes.float8_e4m3

import concourse.bass as bass
import concourse.mybir as mybir
import concourse.tile as tile
from concourse import bacc
from concourse.bass_utils import run_bass_kernel_spmd
from concourse.masks import make_identity

F32 = mybir.dt.float32
BF = mybir.dt.bfloat16
F8 = mybir.dt.float8e4
AF = mybir.ActivationFunctionType
ALU = mybir.AluOpType
DR = mybir.MatmulPerfMode.DoubleRow

V, C, T, H, L, B = 32000, 1024, 1024, 16, 4, 4
HD = C // H          # 64
FF = 4 * C           # 4096
NCORES = 8
TL = 512             # local tokens per core (4 seqs x 128)
SEQ = B              # 4
NTG = 8              # t-groups of 512 over the full 4096 tokens
VSH = V // NCORES    # 4000
VCW = 500            # vocab chunk (<=512 psum)
LN_EPS = 1e-5
WS = 64.0            # weight fp8 scale for k/v/w1/w2/wlm
QS = 8.0             # wq carries 64/8 (HD^-0.5 folded)
OS = 8.0             # o fp8 scale (via V pad), wo carries 8x

HT_SZ = C * TL            # fp8 h^T shard elems (512KB)
O_SZ = NCORES * TL * 128  # o alltoall buffer elems (512KB fp8)

_prog_cache = {}


def _ap(t, offset, pattern):
    return bass.AP(tensor=t.tensor if isinstance(t, bass.AP) else t, offset=offset, ap=pattern)


def _build(LL=L, debug=False, sim=False):
    key = (LL, debug, sim)
    if key in _prog_cache:
        return _prog_cache[key]

    nc = bacc.Bacc("TRN2", target_bir_lowering=False, debug=False, num_devices=NCORES)

    x0_d = nc.dram_tensor("x0", [TL, C], F32, kind="ExternalInput")
    tril_d = nc.dram_tensor("tril", [128, 128], BF, kind="ExternalInput")
    wq_d = nc.dram_tensor("wq", [L, 4, 128, 2, 128], BF, kind="ExternalInput")
    wk_d = nc.dram_tensor("wk", [L, 4, 128, 2, 128], BF, kind="ExternalInput")
    wv_d = nc.dram_tensor("wv", [L, 4, 128, 2, 128], BF, kind="ExternalInput")
    qb_d = nc.dram_tensor("qb", [L, 128], F32, kind="ExternalInput")
    kb_d = nc.dram_tensor("kb", [L, 128], F32, kind="ExternalInput")
    vb_d = nc.dram_tensor("vb", [L, 128], F32, kind="ExternalInput")  # per-core d-slice
    wo_d = nc.dram_tensor("wo", [L, 4, 128, 2, C], BF, kind="ExternalInput")
    bo_d = nc.dram_tensor("bo", [L, C], F32, kind="ExternalInput")
    w1_d = nc.dram_tensor("w1", [L, 4, 128, 2, FF], BF, kind="ExternalInput")
    b1g_d = nc.dram_tensor("b1g", [L, 128, 32], F32, kind="ExternalInput")
    w2_d = nc.dram_tensor("w2", [L, 16, 128, 2, C], BF, kind="ExternalInput")
    b2_d = nc.dram_tensor("b2", [L, C], F32, kind="ExternalInput")
    wlm_d = nc.dram_tensor("wlm", [4, 128, 2, VSH], BF, kind="ExternalInput")

    logits_d = nc.dram_tensor("logits", [NCORES, SEQ, 128, VSH], BF, kind="ExternalOutput")
    dbg_d = None
    if debug:
        dbg_d = nc.dram_tensor("dbg", [LL, TL, C], F32, kind="ExternalOutput")

    with tile.TileContext(nc) as tc:
        import contextlib

        with contextlib.ExitStack() as ctx:
            const = ctx.enter_context(tc.tile_pool(name="const", bufs=1))
            xpool = ctx.enter_context(tc.tile_pool(name="x", bufs=1))
            hpool = ctx.enter_context(tc.tile_pool(name="h", bufs=4))
            htpool = ctx.enter_context(tc.tile_pool(name="hT", bufs=1))
            htf_pool = ctx.enter_context(tc.tile_pool(name="hTf", bufs=1))
            wsm_pool = ctx.enter_context(tc.tile_pool(name="wsm", bufs=1))
            wbig_pool = ctx.enter_context(tc.tile_pool(name="wbig", bufs=1))
            qkpool = ctx.enter_context(tc.tile_pool(name="qk", bufs=1))
            vpool = ctx.enter_context(tc.tile_pool(name="v", bufs=1))
            ptpool = ctx.enter_context(tc.tile_pool(name="pt", bufs=2))
            opool = ctx.enter_context(tc.tile_pool(name="o", bufs=1))
            otpool = ctx.enter_context(tc.tile_pool(name="ot", bufs=1))
            ugpool = ctx.enter_context(tc.tile_pool(name="ug", bufs=1))
            bpool = ctx.enter_context(tc.tile_pool(name="b", bufs=1))
            misc = ctx.enter_context(tc.tile_pool(name="misc", bufs=2))
            lgpool = ctx.enter_context(tc.tile_pool(name="lg", bufs=1))
            hfr_pool = ctx.enter_context(tc.tile_pool(name="hfr", bufs=2))
            ps_acc = ctx.enter_context(tc.tile_pool(name="psacc", bufs=4, space="PSUM"))
            ps_st = ctx.enter_context(tc.tile_pool(name="psst", bufs=2, space="PSUM"))
            ps_pv = ctx.enter_context(tc.tile_pool(name="pspv", bufs=2, space="PSUM"))
            dram = ctx.enter_context(tc.tile_pool(name="dram", bufs=1, space="DRAM"))

            ident = const.tile([128, 128], BF, name="ident")
            make_identity(nc, ident)
            eps_t = const.tile([128, 1], F32, name="eps")
            nc.vector.memset(eps_t[:], LN_EPS)
            tril_t = const.tile([128, 128], BF, name="tril")
            nc.sync.dma_start(out=tril_t[:], in_=tril_d[:])
            qb_t = const.tile([128, L], F32, name="qb_t")
            nc.sync.dma_start(out=qb_t[:], in_=_ap(qb_d, 0, [[1, 128], [128, L]]))
            kb_t = const.tile([128, L], F32, name="kb_t")
            nc.sync.dma_start(out=kb_t[:], in_=_ap(kb_d, 0, [[1, 128], [128, L]]))

            # persistent fp32 residual stream; tile tt = seq tt, rows 128c..
            x_t = [xpool.tile([128, C], F32, tag=f"x{tt}", name=f"x{tt}") for tt in range(SEQ)]
            for tt in range(SEQ):
                nc.sync.dma_start(out=x_t[tt][:], in_=x0_d[tt * 128:(tt + 1) * 128, :])

            def bcast_row(dst, src_tensor, offset, n):
                src = _ap(src_tensor, offset, [[0, dst.shape[0]], [1, n]])
                nc.gpsimd.dma_start(out=dst[:], in_=src)

            U32 = mybir.dt.uint32

            def emit_ln(tag):
                """fp32 x_t -> fp8 normalized (x-m)*rstd, transposed into
                c-paired tiles hT[j][ki, ko, t] = h[t, (2j+ko)*128+ki].
                rstd comes from a DVE-only fast inverse sqrt (magic-constant
                seed + 2 Newton steps) to keep Ln/Sqrt off the ACT tables."""
                mv_all = misc.tile([128, SEQ, 2], F32, name="mv_all", tag="mv")
                for tt in range(SEQ):
                    stats = misc.tile([128, 2, 6], F32, name="stats", tag="stats")
                    xv = x_t[tt][:].rearrange("p (s d) -> p s d", s=2)
                    nc.vector.bn_stats(out=stats[:, 0, :], in_=xv[:, 0, :])
                    nc.vector.bn_stats(out=stats[:, 1, :], in_=xv[:, 1, :])
                    nc.vector.bn_aggr(out=mv_all[:, tt, :], in_=stats[:])
                vv = misc.tile([128, SEQ], F32, name="vv", tag="vv")
                nc.vector.tensor_scalar(
                    out=vv[:], in0=mv_all[:, :, 1], scalar1=LN_EPS, scalar2=None,
                    op0=ALU.add)
                y = misc.tile([128, SEQ], F32, name="rsq", tag="rsq")
                nc.vector.tensor_scalar(
                    out=y[:].bitcast(U32), in0=vv[:].bitcast(U32),
                    scalar1=1, scalar2=None, op0=ALU.logical_shift_right)
                nc.vector.tensor_scalar(
                    out=y[:].bitcast(U32), in0=y[:].bitcast(U32),
                    scalar1=-1, scalar2=0x5F3759DF, op0=ALU.mult, op1=ALU.add)
                hv = misc.tile([128, SEQ], F32, name="hv", tag="hv")
                nc.vector.tensor_scalar(
                    out=hv[:], in0=vv[:], scalar1=-0.5, scalar2=None, op0=ALU.mult)
                y2 = misc.tile([128, SEQ], F32, name="y2", tag="y2")
                for _ in range(2):
                    nc.vector.tensor_mul(out=y2[:], in0=y[:], in1=y[:])
                    nc.vector.tensor_mul(out=y2[:], in0=y2[:], in1=hv[:])
                    nc.vector.tensor_scalar(
                        out=y2[:], in0=y2[:], scalar1=1.5, scalar2=None, op0=ALU.add)
                    nc.vector.tensor_mul(out=y[:], in0=y[:], in1=y2[:])
                h_tiles = []
                for tt in range(SEQ):
                    h = hpool.tile([128, C], BF, tag="h", name="h")
                    nc.vector.tensor_scalar(
                        out=h[:], in0=x_t[tt][:], scalar1=mv_all[:, tt, 0:1],
                        scalar2=y[:, tt:tt + 1],
                        op0=ALU.subtract, op1=ALU.mult,
                    )
                    h_tiles.append(h)
                hT = []
                for j in range(4):
                    pst = ps_st.tile([128, 2, 512], BF, tag="st", name="pst")
                    for ko in range(2):
                        cb = 2 * j + ko
                        for tt in range(SEQ):
                            nc.tensor.transpose(
                                pst[:, ko, tt * 128:(tt + 1) * 128],
                                h_tiles[tt][:, cb * 128:(cb + 1) * 128],
                                ident[:],
                            )
                    ht = htpool.tile([128, 2, 512], BF, tag=f"{tag}{j}", name=f"{tag}{j}")
                    if j % 2 == 0:
                        nc.scalar.activation(ht[:], pst[:], AF.Copy)
                    else:
                        nc.vector.tensor_copy(out=ht[:], in_=pst[:])
                    hT.append(ht)
                return hT

            for l in range(LL):
                lw = l % L
                # ---------- LN1 + local h^T ----------
                hT = emit_ln("hT")
                # x += bo early (Pool), off the critical path: LN1 already
                # consumed x, and the reference adds bo before LN2.
                bo_b = bpool.tile([128, C], F32, tag="bb", name="bo_b")
                bcast_row(bo_b, bo_d, lw * C, C)
                for tt in range(SEQ):
                    nc.gpsimd.tensor_tensor(
                        out=x_t[tt][:], in0=x_t[tt][:], in1=bo_b[:], op=ALU.add)
                hT_loc = dram.tile([HT_SZ], BF, tag="ht_loc", name="ht_loc")
                for j in range(4):
                    # [ki, ko, t] -> row (2j+ko)*128+ki, col t
                    nc.sync.dma_start(
                        out=_ap(hT_loc, 2 * j * 128 * TL,
                                [[TL, 128], [128 * TL, 2], [1, TL]]),
                        in_=hT[j][:],
                    )
                hT_full = dram.tile([NCORES * HT_SZ], BF,
                                    addr_space="Local" if sim else "Shared",
                                    tag=f"htf{l}", name=f"htf{l}")
                if sim:
                    nc.sync.dma_start(
                        out=_ap(hT_full, 0, [[2048, HT_SZ // 2048], [1, 2048]]),
                        in_=_ap(hT_loc, 0, [[2048, HT_SZ // 2048], [1, 2048]]),
                    )
                else:
                    nc.gpsimd.collective_compute(
                        "AllGather",
                        ALU.bypass,
                        replica_groups=[list(range(NCORES))],
                        ins=[_ap(hT_loc, 0, [[2048, HT_SZ // 2048], [1, 2048]])],
                        outs=[_ap(hT_full, 0, [[2048, NCORES * HT_SZ // 2048], [1, 2048]])],
                    )

                # ---------- QKV for my 2 heads over all 4096 tokens ----------
                # DRAM layout is [j, ki, ko, d]; SBUF tile is [ki, j, ko, d]
                def _w_ap(wd, base, nj, dcols):
                    return _ap(wd, base, [[2 * dcols, 128], [128 * 2 * dcols, nj],
                                          [dcols, 2], [1, dcols]])
                wq_t = wsm_pool.tile([128, 4, 2, 128], BF, tag="wq", name="wq_t")
                nc.sync.dma_start(out=wq_t[:], in_=_w_ap(wq_d, lw * 131072, 4, 128))
                wk_t = wsm_pool.tile([128, 4, 2, 128], BF, tag="wk", name="wk_t")
                nc.sync.dma_start(out=wk_t[:], in_=_w_ap(wk_d, lw * 131072, 4, 128))
                wv_t = wsm_pool.tile([128, 4, 2, 128], BF, tag="wv", name="wv_t")
                nc.sync.dma_start(out=wv_t[:], in_=_w_ap(wv_d, lw * 131072, 4, 128))

                vb_b = bpool.tile([128, 128], F32, tag="vb", name="vb_b")
                bcast_row(vb_b, vb_d, lw * 128, 128)
                qT = []  # 8 tiles [128 d(2h), 512 t] bf16
                kT = qkpool.tile([128, NTG, 512], BF, tag="kT", name="kT")
                v_sb = vpool.tile([128, 32, 2, HD + 1], BF, tag="v", name="v_sb")
                nc.vector.memset(v_sb[:, :, :, HD:HD + 1], OS)
                for tg in range(NTG):
                    htf = htf_pool.tile([128, 4, 2, 512], BF, tag="htf", name="htf")
                    for jh in range(2):
                        nc.sync.dma_start(
                            out=htf[:, jh * 2:(jh + 1) * 2, :, :],
                            in_=_ap(hT_full, tg * HT_SZ + jh * 2 * 2 * 128 * TL,
                                    [[TL, 128], [2 * 128 * TL, 2], [128 * TL, 2], [1, 512]]),
                        )
                    # Q^T, K^T: out [128 d, 512 t]
                    psq = ps_acc.tile([128, 512], F32, tag="acc", name="psq")
                    psk = ps_acc.tile([128, 512], F32, tag="acc", name="psk")
                    for j in range(4):
                        for ko in range(2):
                            nc.tensor.matmul(
                                psq[:], wq_t[:, j, ko, :], htf[:, j, ko, :],
                                start=(j == 0 and ko == 0), stop=(j == 3 and ko == 1),
                            )
                    for j in range(4):
                        for ko in range(2):
                            nc.tensor.matmul(
                                psk[:], wk_t[:, j, ko, :], htf[:, j, ko, :],
                                start=(j == 0 and ko == 0), stop=(j == 3 and ko == 1),
                            )
                    qt = qkpool.tile([128, 512], BF, tag=f"qT{tg}", name=f"qT{tg}")
                    nc.vector.tensor_scalar(
                        out=qt[:], in0=psq[:], scalar1=qb_t[:, lw:lw + 1],
                        scalar2=None, op0=ALU.add)
                    qT.append(qt)
                    nc.vector.tensor_scalar(
                        out=kT[:, tg, :], in0=psk[:], scalar1=kb_t[:, lw:lw + 1],
                        scalar2=None, op0=ALU.add)
                    # V natural: out [128 t, 128 d] per tt; cols 128 = (s=t4,j) tokens
                    psv = ps_acc.tile([128, 4, 128], F32, tag="acc", name="psv")
                    for t4 in range(4):
                        for j in range(4):
                            for ko in range(2):
                                nc.tensor.matmul(
                                    psv[:, t4, :],
                                    htf[:, j, ko, t4 * 128:(t4 + 1) * 128],
                                    wv_t[:, j, ko, :],
                                    start=(j == 0 and ko == 0),
                                    stop=(j == 3 and ko == 1),
                                )
                    for t4 in range(4):
                        nc.vector.tensor_tensor(
                            out=v_sb[:, tg * 4 + t4, :, 0:HD],
                            in0=psv[:, t4, :].rearrange("p (b d) -> p b d", b=2),
                            in1=vb_b[:].rearrange("p (b d) -> p b d", b=2),
                            op=ALU.add,
                        )

                # ---------- causal attention (my 2 heads, all seqs) ----------
                o_loc = dram.tile([O_SZ], BF, tag="o_loc", name="o_loc")
                for s in range(SEQ):
                    o_st = opool.tile([128, 8, 2, HD], BF, tag="ost", name=f"ost{s}")
                    for hp in range(2):
                        p0 = hp * HD
                        po = None
                        recip = None
                        for qt in range(8):
                            # q-tile (s, qt) lives at gathered cols qt*512 + s*128 + j
                            qsl = qT[qt][p0:p0 + HD, s * 128:(s + 1) * 128]
                            nk = qt + 1
                            pt = ptpool.tile([128, 8, 128], BF, tag="pt", name="pt")
                            for half in range((nk + 3) // 4):
                                k0 = half * 4
                                k1 = min(nk, k0 + 4)
                                st = ps_st.tile([128, 4, 128], F32, tag="st", name="st")
                                for kt in range(k0, k1):
                                    nc.tensor.matmul(
                                        st[:, kt - k0, :],
                                        kT[p0:p0 + HD, kt, s * 128:(s + 1) * 128],
                                        qsl,
                                        start=True, stop=True,
                                    )
                                nc.scalar.activation(
                                    pt[:, k0:k1, :], st[:, 0:k1 - k0, :],
                                    AF.Exp, bias=0.0, scale=1.0 / 4096.0,
                                )
                            # causal mask on the diagonal tile only
                            nc.vector.tensor_mul(
                                out=pt[:, qt, :], in0=pt[:, qt, :], in1=tril_t[:])
                            if qt % 4 == 0:
                                po = ps_pv.tile([128, 4, HD + 1], F32, tag="pv", name="po")
                            for kt in range(nk):
                                nc.tensor.matmul(
                                    po[:, qt % 4, :],
                                    pt[:, kt, :],
                                    v_sb[:, kt * 4 + s, hp, :],
                                    start=(kt == 0), stop=(kt == nk - 1),
                                )
                            if qt % 4 == 3:
                                recip = misc.tile([128, 4, 1], F32, tag="recip", name="recip")
                                nc.vector.reciprocal(recip[:], po[:, :, HD:HD + 1])
                                for q2 in range(qt - 3, qt + 1):
                                    nc.vector.tensor_scalar(
                                        out=o_st[:, q2, hp, :], in0=po[:, q2 % 4, 0:HD],
                                        scalar1=recip[:, q2 % 4, :], scalar2=None,
                                        op0=ALU.mult,
                                    )
                    # rows of o_loc block d: (s,j); cols (hp, dd)
                    nc.sync.dma_start(
                        out=_ap(o_loc, s * 128 * 128,
                                [[128, 128], [TL * 128, 8], [HD, 2], [1, HD]]),
                        in_=o_st[:],
                    )

                # prefetch wo / w1 / w2 while attention finishes (1MB chunks so
                # the exclusive DMA device is never held long)
                wo_t = wbig_pool.tile([128, 4, 2, C], BF, tag="w2", name="wo_t")
                nc.sync.dma_start(out=wo_t[:], in_=_w_ap(wo_d, lw * 4 * 256 * C, 4, C))

                o_x = dram.tile([O_SZ], BF, tag="o_x", name="o_x")
                if sim:
                    nc.sync.dma_start(
                        out=_ap(o_x, 0, [[2048, O_SZ // 2048], [1, 2048]]),
                        in_=_ap(o_loc, 0, [[2048, O_SZ // 2048], [1, 2048]]),
                    )
                else:
                    nc.gpsimd.collective_compute(
                        "AllToAll",
                        ALU.bypass,
                        replica_groups=[list(range(NCORES))],
                        ins=[_ap(o_loc, 0, [[TL * 128, NCORES], [2048, 32], [1, 2048]])],
                        outs=[_ap(o_x, 0, [[TL * 128, NCORES], [2048, 32], [1, 2048]])],
                    )

                # read back my tokens' attention output, transpose to c-pairs
                OT = []
                o_rd = vpool.tile([128, SEQ, 8, 2, HD], BF, tag="v", name="ord")
                for s in range(SEQ):
                    nc.sync.dma_start(
                        out=o_rd[:, s, :, :, :],
                        in_=_ap(o_x, s * 128 * 128,
                                [[128, 128], [TL * 128, 8], [HD, 2], [1, HD]]),
                    )
                for j in range(4):
                    pst = ps_st.tile([128, 2, 512], BF, tag="st", name="pst_o")
                    for ko in range(2):
                        cb = 2 * j + ko
                        for s in range(SEQ):
                            nc.tensor.transpose(
                                pst[:, ko, s * 128:(s + 1) * 128],
                                o_rd[:, s, cb, :, :].rearrange("p a b -> p (a b)"),
                                ident[:],
                            )
                    ot = otpool.tile([128, 2, 512], BF, tag=f"OT{j}", name=f"OT{j}")
                    if j % 2 == 0:
                        nc.scalar.activation(ot[:], pst[:], AF.Copy)
                    else:
                        nc.vector.tensor_copy(out=ot[:], in_=pst[:])
                    OT.append(ot)

                # ---------- Wo + residual ----------
                bo_b = bpool.tile([128, C], F32, tag="bb", name="bo_b")
                bcast_row(bo_b, bo_d, lw * C, C)
                for tt in range(SEQ):
                    for nf in range(2):
                        ps = ps_acc.tile([128, 512], F32, tag="acc", name="pso")
                        for j in range(4):
                            for ko in range(2):
                                nc.tensor.matmul(
                                    ps[:], OT[j][:, ko, tt * 128:(tt + 1) * 128],
                                    wo_t[:, j, ko, nf * 512:(nf + 1) * 512],
                                    start=(j == 0 and ko == 0),
                                    stop=(j == 3 and ko == 1),
                                )
                        xs = x_t[tt][:, nf * 512:(nf + 1) * 512]
                        nc.vector.scalar_tensor_tensor(
                            out=xs, in0=ps[:], scalar=1.0 / 64.0, in1=xs,
                            op0=ALU.mult, op1=ALU.add)
                    nc.gpsimd.tensor_tensor(
                        out=x_t[tt][:], in0=x_t[tt][:], in1=bo_b[:], op=ALU.add)

                # ---------- LN2 + FFN (bf16, W1/W2 streamed in halves) ----------
                h2T = emit_ln("hT")  # reuse tags; local tokens only
                b1g_t = misc.tile([128, 32], F32, tag="b1g", name="b1g_t")
                nc.sync.dma_start(out=b1g_t[:], in_=b1g_d[lw])
                ug = ugpool.tile([128, 32, 512], BF, tag="ug", name="ug")
                for fh in range(2):
                    w1_t = wbig_pool.tile([128, 4, 2, FF // 2], BF, tag="w1", name="w1_t")
                    for j in range(4):
                        nc.sync.dma_start(
                            out=w1_t[:, j, :, :],
                            in_=_ap(w1_d, lw * 4 * 256 * FF + j * 128 * 2 * FF + fh * (FF // 2),
                                    [[2 * FF, 128], [FF, 2], [1, FF // 2]]))
                    for fg in range(4):
                        pss = [ps_acc.tile([128, 512], F32, tag="acc", name="psf")
                               for _ in range(4)]
                        for f4 in range(4):
                            fb = fg * 4 + f4
                            for j in range(4):
                                for ko in range(2):
                                    nc.tensor.matmul(
                                        pss[f4][:],
                                        w1_t[:, j, ko, fb * 128:(fb + 1) * 128],
                                        h2T[j][:, ko, :],
                                        start=(j == 0 and ko == 0),
                                        stop=(j == 3 and ko == 1),
                                    )
                        for f4 in range(4):
                            fb = fh * 16 + fg * 4 + f4
                            nc.scalar.activation(
                                ug[:, fb, :], pss[f4][:], AF.Gelu,
                                bias=b1g_t[:, fb:fb + 1], scale=1.0 / 64.0,
                            )
                b2_b = bpool.tile([128, C], F32, tag="bb", name="b2_b")
                bcast_row(b2_b, b2_d, lw * C, C)
                for nf in range(2):
                    w2_t = wbig_pool.tile([128, 16, 2, 512], BF, tag="w2", name="w2_t")
                    for mg in range(4):
                        for mi in range(4):
                            mm = mg * 4 + mi
                            nc.sync.dma_start(
                                out=w2_t[:, mm, :, :],
                                in_=_ap(w2_d, lw * 16 * 256 * C + mm * 256 * C + nf * 512,
                                        [[2 * C, 128], [C, 2], [1, 512]]))
                    for tt in range(SEQ):
                        ps = ps_acc.tile([128, 512], F32, tag="acc", name="ps2")
                        for m in range(16):
                            for ko in range(2):
                                nc.tensor.matmul(
                                    ps[:], ug[:, 2 * m + ko, tt * 128:(tt + 1) * 128],
                                    w2_t[:, m, ko, :],
                                    start=(m == 0 and ko == 0),
                                    stop=(m == 15 and ko == 1),
                                )
                        xs = x_t[tt][:, nf * 512:(nf + 1) * 512]
                        nc.vector.scalar_tensor_tensor(
                            out=xs, in0=ps[:], scalar=1.0 / 64.0, in1=xs,
                            op0=ALU.mult, op1=ALU.add)
                for tt in range(SEQ):
                    nc.gpsimd.tensor_tensor(
                        out=x_t[tt][:], in0=x_t[tt][:], in1=b2_b[:], op=ALU.add)

                if debug:
                    for tt in range(SEQ):
                        nc.sync.dma_start(
                            out=dbg_d[l, tt * 128:(tt + 1) * 128, :], in_=x_t[tt][:]
                        )

            # ---------- final LN, AllGather hf^T, lm_head ----------

            hfT = emit_ln("hT")
            hfT_loc = dram.tile([HT_SZ], BF, tag="hf_loc", name="hf_loc")
            for j in range(4):
                nc.sync.dma_start(
                    out=_ap(hfT_loc, 2 * j * 128 * TL,
                            [[TL, 128], [128 * TL, 2], [1, TL]]),
                    in_=hfT[j][:],
                )
            hfT_full = dram.tile([NCORES * HT_SZ], BF,
                                 addr_space="Local" if sim else "Shared",
                                 tag="hff", name="hff")
            if sim:
                nc.sync.dma_start(
                    out=_ap(hfT_full, 0, [[2048, HT_SZ // 2048], [1, 2048]]),
                    in_=_ap(hfT_loc, 0, [[2048, HT_SZ // 2048], [1, 2048]]),
                )
            else:
                nc.gpsimd.collective_compute(
                    "AllGather",
                    ALU.bypass,
                    replica_groups=[list(range(NCORES))],
                    ins=[_ap(hfT_loc, 0, [[2048, HT_SZ // 2048], [1, 2048]])],
                    outs=[_ap(hfT_full, 0, [[2048, NCORES * HT_SZ // 2048], [1, 2048]])],
                )

            VH = VSH // 2  # 2000
            for vh in range(2):
                wlm_t = wbig_pool.tile([128, 4, 2, VH], BF, tag="w1", name="wlm_t")
                for j in range(4):
                    nc.sync.dma_start(
                        out=wlm_t[:, j, :, :],
                        in_=_ap(wlm_d, j * 128 * 2 * VSH + vh * VH,
                                [[2 * VSH, 128], [VSH, 2], [1, VH]]))
                for r in range(NCORES):
                    hfr = hfr_pool.tile([128, 4, 2, 512], BF, tag="hfr", name="hfr")
                    nc.sync.dma_start(
                        out=hfr[:],
                        in_=_ap(hfT_full, r * HT_SZ,
                                [[TL, 128], [2 * 128 * TL, 4], [128 * TL, 2], [1, TL]]),
                    )
                    for ts in range(SEQ):
                        for half in range(2):
                            lg = lgpool.tile([128, VH // 2], BF,
                                             tag=f"lg{half}", name=f"lg{half}")
                            for v2 in range(2):
                                vc = half * 2 + v2
                                ps = ps_acc.tile([128, VCW], F32, tag="acc", name="psl")
                                for j in range(4):
                                    for ko in range(2):
                                        nc.tensor.matmul(
                                            ps[:],
                                            hfr[:, j, ko, ts * 128:(ts + 1) * 128],
                                            wlm_t[:, j, ko, vc * VCW:(vc + 1) * VCW],
                                            start=(j == 0 and ko == 0),
                                            stop=(j == 3 and ko == 1),
                                        )
                                if vc % 2 == 0:
                                    nc.scalar.activation(
                                        lg[:, v2 * VCW:(v2 + 1) * VCW], ps[:], AF.Copy)
                                else:
                                    nc.vector.tensor_copy(
                                        out=lg[:, v2 * VCW:(v2 + 1) * VCW], in_=ps[:])
                            nc.sync.dma_start(
                                out=logits_d[r, ts, :,
                                             vh * VH + half * (VH // 2):
                                             vh * VH + (half + 1) * (VH // 2)],
                                in_=lg[:])

    nc.compile()
    _prog_cache[key] = nc
    return nc


def _q8(a, scale):
    return np.ascontiguousarray(np.asarray(a, np.float32) * scale).astype(BF16NP)


def _pair_w(w, scale, g=None):
    """[K, D] -> [K/256, 128, 2, D] fp8 with k = (2j+ko)*128+ki, opt. row gain."""
    wf = np.asarray(w, np.float32)
    if g is not None:
        wf = wf * np.asarray(g, np.float32)[:, None]
    k, d = wf.shape
    return _q8(wf.reshape(k // 256, 2, 128, d).transpose(0, 2, 1, 3), scale)


def _prep_inputs(inputs):
    f = {k: np.asarray(k_v) for k, k_v in inputs.items()}
    idx = f["idx"].astype(np.int64)
    emb = f["emb"].astype(np.float32)
    pos = f["pos_enc"].astype(np.float32)
    x_full = emb[idx] + pos[None, :, :]          # [B, T, C] f32

    Wq, Wk, Wv, Wo = (f[k].astype(np.float32) for k in ("Wq", "Wk", "Wv", "Wo"))
    W1, W2 = f["W1"].astype(np.float32), f["W2"].astype(np.float32)
    g1, b1n = f["ln1_g"].astype(np.float32), f["ln1_b"].astype(np.float32)
    g2, b2n = f["ln2_g"].astype(np.float32), f["ln2_b"].astype(np.float32)
    gf, bfn = f["lnf_g"].astype(np.float32), f["lnf_b"].astype(np.float32)
    b1 = f["b1"].astype(np.float32)
    bo, b2v = f["bo"].astype(np.float32), f["b2"].astype(np.float32)
    Wlm = f["Wlm"].astype(np.float32)
    blm = f["blm"].astype(np.float32)

    # layer-folded full tensors
    wq_l, wk_l, wv_l = [], [], []
    qb_l, kb_l, vb_l = [], [], []
    wo_l, w1_l, w2_l, b1g_l = [], [], [], []
    for l in range(L):
        wq_l.append(_pair_w(Wq[l], QS, g1[l]))          # [4,128,2,C]
        wk_l.append(_pair_w(Wk[l], WS, g1[l]))
        wv_l.append(_pair_w(Wv[l], WS, g1[l]))
        qb_l.append(QS * (b1n[l] @ Wq[l]))
        kb_l.append(WS * (b1n[l] @ Wk[l]))
        vb_l.append(WS * (b1n[l] @ Wv[l]))
        wo_l.append(_pair_w(Wo[l], OS))
        w1_l.append(_pair_w(W1[l], WS, g2[l]))
        b1p = b1[l] + b2n[l] @ W1[l]
        b1g_l.append(np.ascontiguousarray(b1p.reshape(32, 128).T, np.float32))
        w2_l.append(_pair_w(W2[l], WS))
    wlm_full = _pair_w(Wlm, WS, gf)                      # [4,128,2,V]
    blm_adj = blm + bfn @ Wlm

    tril = np.ascontiguousarray(
        (np.arange(128)[None, :] >= np.arange(128)[:, None]).astype(np.float32)
    ).astype(BF16NP)

    shared = {
        "tril": tril,
        "bo": bo, "b2": b2v,
        "b1g": np.stack(b1g_l),
        "w1": np.stack(w1_l), "w2": np.stack(w2_l),
        "wo": np.stack(wo_l),
    }

    in_maps = []
    for c in range(NCORES):
        im = dict(shared)
        cs = slice(c * 128, (c + 1) * 128)
        im["x0"] = np.ascontiguousarray(
            x_full[:, cs, :].reshape(TL, C), dtype=np.float32)
        im["wq"] = np.stack([w[:, :, :, cs] for w in wq_l])
        im["wk"] = np.stack([w[:, :, :, cs] for w in wk_l])
        im["wv"] = np.stack([w[:, :, :, cs] for w in wv_l])
        im["qb"] = np.stack([q[cs] for q in qb_l])
        im["kb"] = np.stack([q[cs] for q in kb_l])
        im["vb"] = np.stack([q[cs] for q in vb_l])
        im["wlm"] = np.ascontiguousarray(wlm_full[:, :, :, c * VSH:(c + 1) * VSH])
        in_maps.append(im)
    return in_maps, blm_adj


def kernel(**inputs):
    nc = _build()
    in_maps, blm_adj = _prep_inputs(inputs)
    res = run_bass_kernel_spmd(nc, in_maps, list(range(NCORES)))
    full = np.empty((B, T, V), np.float32)
    for c in range(NCORES):
        part = np.asarray(res.results[c]["logits"], dtype=np.float32)  # [8,4,128,VSH]
        full[:, :, c * VSH:(c + 1) * VSH] = (
            part.transpose(1, 0, 2, 3).reshape(B, T, VSH))
    full = full * (1.0 / 64.0) + blm_adj[None, None, :]
    return np.ascontiguousarray(full, dtype=np.float32)


# revision 44
# speedup vs baseline: 1.0271x; 1.0271x over previous
"""GPT forward pass on 8 Trainium2 NeuronCores.

Sharding: token-parallel trunk (core c owns rows 128c..128c+127 of each of
the 4 sequences = 512 tokens), head-parallel attention (core c owns heads
2c, 2c+1 over ALL tokens -- causal triangular work is then uniform across
cores, so SPMD loop bounds can skip the upper triangle), vocab-sharded
lm_head (4000 cols/core).

Per layer: LN1 -> AllGather h^T (bf16, 1MB) -> QKV for my 2 heads over all
4096 tokens (K/V stay SBUF-resident, no DRAM KV) -> causal scores/softmax/PV
-> AllToAll O back to token shards (bf16, 1MB) -> Wo + residual -> LN2 ->
FFN -> residual.

All matmuls run bf16 with fp32 PSUM (fp8 was tried and fails the 2e-2
accuracy gate: each e4m3 GEMM contributes ~3e-2 relative error). LayerNorm
gains/biases are folded into the following weights on the host; rstd is a
DVE-only fast inverse sqrt (no ACT table thrash). Weights keep power-of-2
scales (8x/64x) inherited from the fp8 design (harmless in bf16), descaled
via free scale slots (exp scale 1/4096, gelu scale 1/64, pad-column 8 for
softmax denominators). Residual stream is fp32. Logits leave the device as
bf16 at 64x scale; the host descales and adds blm (+ lnf_b @ Wlm).
"""

import os
import sys

for _p in ("/opt/trn_rl_repo",):
    if os.path.isdir(_p) and _p not in sys.path:
        sys.path.insert(0, _p)

import numpy as np
import ml_dtypes

BF16NP = ml_dtypes.bfloat16
F8NP = ml_dtypes.float8_e4m3

import concourse.bass as bass
import concourse.mybir as mybir
import concourse.tile as tile
from concourse import bacc
from concourse.bass_utils import run_bass_kernel_spmd
from concourse.masks import make_identity

F32 = mybir.dt.float32
BF = mybir.dt.bfloat16
F8 = mybir.dt.float8e4
AF = mybir.ActivationFunctionType
ALU = mybir.AluOpType
DR = mybir.MatmulPerfMode.DoubleRow

V, C, T, H, L, B = 32000, 1024, 1024, 16, 4, 4
HD = C // H          # 64
FF = 4 * C           # 4096
NCORES = 8
TL = 512             # local tokens per core (4 seqs x 128)
SEQ = B              # 4
NTG = 8              # t-groups of 512 over the full 4096 tokens
VSH = V // NCORES    # 4000
VCW = 500            # vocab chunk (<=512 psum)
LN_EPS = 1e-5
WS = 64.0            # weight fp8 scale for k/v/w1/w2/wlm
QS = 8.0             # wq carries 64/8 (HD^-0.5 folded)
OS = 8.0             # o fp8 scale (via V pad), wo carries 8x

HT_SZ = C * TL            # fp8 h^T shard elems (512KB)
O_SZ = NCORES * TL * 128  # o alltoall buffer elems (512KB fp8)

_prog_cache = {}


def _ap(t, offset, pattern):
    return bass.AP(tensor=t.tensor if isinstance(t, bass.AP) else t, offset=offset, ap=pattern)


def _build(LL=L, debug=False, sim=False):
    key = (LL, debug, sim)
    if key in _prog_cache:
        return _prog_cache[key]

    nc = bacc.Bacc("TRN2", target_bir_lowering=False, debug=False, num_devices=NCORES)

    x0_d = nc.dram_tensor("x0", [TL, C], F32, kind="ExternalInput")
    tril_d = nc.dram_tensor("tril", [128, 128], BF, kind="ExternalInput")
    wq_d = nc.dram_tensor("wq", [L, 4, 128, 2, 128], BF, kind="ExternalInput")
    wk_d = nc.dram_tensor("wk", [L, 4, 128, 2, 128], BF, kind="ExternalInput")
    wv_d = nc.dram_tensor("wv", [L, 4, 128, 2, 128], BF, kind="ExternalInput")
    qb_d = nc.dram_tensor("qb", [L, 128], F32, kind="ExternalInput")
    kb_d = nc.dram_tensor("kb", [L, 128], F32, kind="ExternalInput")
    vb_d = nc.dram_tensor("vb", [L, 128], F32, kind="ExternalInput")  # per-core d-slice
    wo_d = nc.dram_tensor("wo", [L, 4, 128, 2, C], BF, kind="ExternalInput")
    bo_d = nc.dram_tensor("bo", [L, C], F32, kind="ExternalInput")
    w1_d = nc.dram_tensor("w1", [L, 4, 128, 2, FF], BF, kind="ExternalInput")
    b1g_d = nc.dram_tensor("b1g", [L, 128, 32], F32, kind="ExternalInput")
    w2_d = nc.dram_tensor("w2", [L, 16, 128, 2, C], BF, kind="ExternalInput")
    b2_d = nc.dram_tensor("b2", [L, C], F32, kind="ExternalInput")
    wlm_d = nc.dram_tensor("wlm", [4, 128, 2, VSH], BF, kind="ExternalInput")

    logits_d = nc.dram_tensor("logits", [NCORES, SEQ, 128, VSH], BF, kind="ExternalOutput")
    dbg_d = None
    if debug:
        dbg_d = nc.dram_tensor("dbg", [LL, TL, C], F32, kind="ExternalOutput")

    with tile.TileContext(nc) as tc:
        import contextlib

        with contextlib.ExitStack() as ctx:
            const = ctx.enter_context(tc.tile_pool(name="const", bufs=1))
            xpool = ctx.enter_context(tc.tile_pool(name="x", bufs=1))
            hpool = ctx.enter_context(tc.tile_pool(name="h", bufs=4))
            htpool = ctx.enter_context(tc.tile_pool(name="hT", bufs=1))
            htf_pool = ctx.enter_context(tc.tile_pool(name="hTf", bufs=1))
            wsm_pool = ctx.enter_context(tc.tile_pool(name="wsm", bufs=1))
            wbig_pool = ctx.enter_context(tc.tile_pool(name="wbig", bufs=1))
            qkpool = ctx.enter_context(tc.tile_pool(name="qk", bufs=1))
            vpool = ctx.enter_context(tc.tile_pool(name="v", bufs=1))
            ptpool = ctx.enter_context(tc.tile_pool(name="pt", bufs=2))
            opool = ctx.enter_context(tc.tile_pool(name="o", bufs=1))
            otpool = ctx.enter_context(tc.tile_pool(name="ot", bufs=1))
            ugpool = ctx.enter_context(tc.tile_pool(name="ug", bufs=1))
            bpool = ctx.enter_context(tc.tile_pool(name="b", bufs=1))
            misc = ctx.enter_context(tc.tile_pool(name="misc", bufs=2))
            lgpool = ctx.enter_context(tc.tile_pool(name="lg", bufs=1))
            hfr_pool = ctx.enter_context(tc.tile_pool(name="hfr", bufs=2))
            ps_acc = ctx.enter_context(tc.tile_pool(name="psacc", bufs=4, space="PSUM"))
            ps_st = ctx.enter_context(tc.tile_pool(name="psst", bufs=2, space="PSUM"))
            ps_pv = ctx.enter_context(tc.tile_pool(name="pspv", bufs=2, space="PSUM"))
            dram = ctx.enter_context(tc.tile_pool(name="dram", bufs=1, space="DRAM"))

            ident = const.tile([128, 128], BF, name="ident")
            make_identity(nc, ident)
            eps_t = const.tile([128, 1], F32, name="eps")
            nc.vector.memset(eps_t[:], LN_EPS)
            tril_t = const.tile([128, 128], BF, name="tril")
            nc.sync.dma_start(out=tril_t[:], in_=tril_d[:])
            qb_t = const.tile([128, L], F32, name="qb_t")
            nc.sync.dma_start(out=qb_t[:], in_=_ap(qb_d, 0, [[1, 128], [128, L]]))
            kb_t = const.tile([128, L], F32, name="kb_t")
            nc.sync.dma_start(out=kb_t[:], in_=_ap(kb_d, 0, [[1, 128], [128, L]]))

            # persistent fp32 residual stream; tile tt = seq tt, rows 128c..
            x_t = [xpool.tile([128, C], F32, tag=f"x{tt}", name=f"x{tt}") for tt in range(SEQ)]
            for tt in range(SEQ):
                nc.sync.dma_start(out=x_t[tt][:], in_=x0_d[tt * 128:(tt + 1) * 128, :])

            def bcast_row(dst, src_tensor, offset, n):
                src = _ap(src_tensor, offset, [[0, dst.shape[0]], [1, n]])
                nc.gpsimd.dma_start(out=dst[:], in_=src)

            U32 = mybir.dt.uint32

            def emit_ln(tag):
                """fp32 x_t -> fp8 normalized (x-m)*rstd, transposed into
                c-paired tiles hT[j][ki, ko, t] = h[t, (2j+ko)*128+ki].
                rstd comes from a DVE-only fast inverse sqrt (magic-constant
                seed + 2 Newton steps) to keep Ln/Sqrt off the ACT tables."""
                mv_all = misc.tile([128, SEQ, 2], F32, name="mv_all", tag="mv")
                for tt in range(SEQ):
                    stats = misc.tile([128, 2, 6], F32, name="stats", tag="stats")
                    xv = x_t[tt][:].rearrange("p (s d) -> p s d", s=2)
                    nc.vector.bn_stats(out=stats[:, 0, :], in_=xv[:, 0, :])
                    nc.vector.bn_stats(out=stats[:, 1, :], in_=xv[:, 1, :])
                    nc.vector.bn_aggr(out=mv_all[:, tt, :], in_=stats[:])
                vv = misc.tile([128, SEQ], F32, name="vv", tag="vv")
                nc.vector.tensor_scalar(
                    out=vv[:], in0=mv_all[:, :, 1], scalar1=LN_EPS, scalar2=None,
                    op0=ALU.add)
                y = misc.tile([128, SEQ], F32, name="rsq", tag="rsq")
                nc.vector.tensor_scalar(
                    out=y[:].bitcast(U32), in0=vv[:].bitcast(U32),
                    scalar1=1, scalar2=None, op0=ALU.logical_shift_right)
                nc.vector.tensor_scalar(
                    out=y[:].bitcast(U32), in0=y[:].bitcast(U32),
                    scalar1=-1, scalar2=0x5F3759DF, op0=ALU.mult, op1=ALU.add)
                hv = misc.tile([128, SEQ], F32, name="hv", tag="hv")
                nc.vector.tensor_scalar(
                    out=hv[:], in0=vv[:], scalar1=-0.5, scalar2=None, op0=ALU.mult)
                y2 = misc.tile([128, SEQ], F32, name="y2", tag="y2")
                for _ in range(2):
                    nc.vector.tensor_mul(out=y2[:], in0=y[:], in1=y[:])
                    nc.vector.tensor_mul(out=y2[:], in0=y2[:], in1=hv[:])
                    nc.vector.tensor_scalar(
                        out=y2[:], in0=y2[:], scalar1=1.5, scalar2=None, op0=ALU.add)
                    nc.vector.tensor_mul(out=y[:], in0=y[:], in1=y2[:])
                h_tiles = []
                for tt in range(SEQ):
                    h = hpool.tile([128, C], BF, tag="h", name="h")
                    nc.vector.tensor_scalar(
                        out=h[:], in0=x_t[tt][:], scalar1=mv_all[:, tt, 0:1],
                        scalar2=y[:, tt:tt + 1],
                        op0=ALU.subtract, op1=ALU.mult,
                    )
                    h_tiles.append(h)
                hT = []
                for j in range(4):
                    pst = ps_st.tile([128, 2, 512], BF, tag="st", name="pst")
                    for ko in range(2):
                        cb = 2 * j + ko
                        for tt in range(SEQ):
                            nc.tensor.transpose(
                                pst[:, ko, tt * 128:(tt + 1) * 128],
                                h_tiles[tt][:, cb * 128:(cb + 1) * 128],
                                ident[:],
                            )
                    ht = htpool.tile([128, 2, 512], BF, tag=f"{tag}{j}", name=f"{tag}{j}")
                    if j % 2 == 0:
                        nc.scalar.activation(ht[:], pst[:], AF.Copy)
                    else:
                        nc.vector.tensor_copy(out=ht[:], in_=pst[:])
                    hT.append(ht)
                return hT

            for l in range(LL):
                lw = l % L
                # ---------- LN1 + local h^T ----------
                hT = emit_ln("hT")
                # x += bo early (Pool), off the critical path: LN1 already
                # consumed x, and the reference adds bo before LN2.
                bo_b = bpool.tile([128, C], F32, tag="bb", name="bo_b")
                bcast_row(bo_b, bo_d, lw * C, C)
                for tt in range(SEQ):
                    nc.gpsimd.tensor_tensor(
                        out=x_t[tt][:], in0=x_t[tt][:], in1=bo_b[:], op=ALU.add)
                hT_loc = dram.tile([HT_SZ], BF, tag="ht_loc", name="ht_loc")
                for j in range(4):
                    # [ki, ko, t] -> row (2j+ko)*128+ki, col t
                    nc.sync.dma_start(
                        out=_ap(hT_loc, 2 * j * 128 * TL,
                                [[TL, 128], [128 * TL, 2], [1, TL]]),
                        in_=hT[j][:],
                    )
                hT_full = dram.tile([NCORES * HT_SZ], BF,
                                    addr_space="Local" if sim else "Shared",
                                    tag=f"htf{l}", name=f"htf{l}")
                if sim:
                    nc.sync.dma_start(
                        out=_ap(hT_full, 0, [[2048, HT_SZ // 2048], [1, 2048]]),
                        in_=_ap(hT_loc, 0, [[2048, HT_SZ // 2048], [1, 2048]]),
                    )
                else:
                    nc.gpsimd.collective_compute(
                        "AllGather",
                        ALU.bypass,
                        replica_groups=[list(range(NCORES))],
                        ins=[_ap(hT_loc, 0, [[2048, HT_SZ // 2048], [1, 2048]])],
                        outs=[_ap(hT_full, 0, [[2048, NCORES * HT_SZ // 2048], [1, 2048]])],
                    )

                # ---------- QKV for my 2 heads over all 4096 tokens ----------
                # DRAM layout is [j, ki, ko, d]; SBUF tile is [ki, j, ko, d]
                def _w_ap(wd, base, nj, dcols):
                    return _ap(wd, base, [[2 * dcols, 128], [128 * 2 * dcols, nj],
                                          [dcols, 2], [1, dcols]])
                wq_t = wsm_pool.tile([128, 4, 2, 128], BF, tag="wq", name="wq_t")
                nc.sync.dma_start(out=wq_t[:], in_=_w_ap(wq_d, lw * 131072, 4, 128))
                wk_t = wsm_pool.tile([128, 4, 2, 128], BF, tag="wk", name="wk_t")
                nc.sync.dma_start(out=wk_t[:], in_=_w_ap(wk_d, lw * 131072, 4, 128))
                wv_t = wsm_pool.tile([128, 4, 2, 128], BF, tag="wv", name="wv_t")
                nc.sync.dma_start(out=wv_t[:], in_=_w_ap(wv_d, lw * 131072, 4, 128))

                vb_b = bpool.tile([128, 128], F32, tag="vb", name="vb_b")
                bcast_row(vb_b, vb_d, lw * 128, 128)
                qT = []  # 8 tiles [128 d(2h), 512 t] bf16
                kT = qkpool.tile([128, NTG, 512], BF, tag="kT", name="kT")
                v_sb = vpool.tile([128, 32, 2, HD + 1], BF, tag="v", name="v_sb")
                nc.vector.memset(v_sb[:, :, :, HD:HD + 1], OS)
                for tg in range(NTG):
                    htf = htf_pool.tile([128, 4, 2, 512], BF, tag="htf", name="htf")
                    for jh in range(2):
                        nc.sync.dma_start(
                            out=htf[:, jh * 2:(jh + 1) * 2, :, :],
                            in_=_ap(hT_full, tg * HT_SZ + jh * 2 * 2 * 128 * TL,
                                    [[TL, 128], [2 * 128 * TL, 2], [128 * TL, 2], [1, 512]]),
                        )
                    # Q^T, K^T: out [128 d, 512 t]
                    psq = ps_acc.tile([128, 512], F32, tag="acc", name="psq")
                    psk = ps_acc.tile([128, 512], F32, tag="acc", name="psk")
                    for j in range(4):
                        for ko in range(2):
                            nc.tensor.matmul(
                                psq[:], wq_t[:, j, ko, :], htf[:, j, ko, :],
                                start=(j == 0 and ko == 0), stop=(j == 3 and ko == 1),
                            )
                    for j in range(4):
                        for ko in range(2):
                            nc.tensor.matmul(
                                psk[:], wk_t[:, j, ko, :], htf[:, j, ko, :],
                                start=(j == 0 and ko == 0), stop=(j == 3 and ko == 1),
                            )
                    qt = qkpool.tile([128, 512], BF, tag=f"qT{tg}", name=f"qT{tg}")
                    nc.vector.tensor_scalar(
                        out=qt[:], in0=psq[:], scalar1=qb_t[:, lw:lw + 1],
                        scalar2=None, op0=ALU.add)
                    qT.append(qt)
                    nc.vector.tensor_scalar(
                        out=kT[:, tg, :], in0=psk[:], scalar1=kb_t[:, lw:lw + 1],
                        scalar2=None, op0=ALU.add)
                    # V natural: out [128 t, 128 d] per tt; cols 128 = (s=t4,j) tokens
                    psv = ps_acc.tile([128, 4, 128], F32, tag="acc", name="psv")
                    for t4 in range(4):
                        for j in range(4):
                            for ko in range(2):
                                nc.tensor.matmul(
                                    psv[:, t4, :],
                                    htf[:, j, ko, t4 * 128:(t4 + 1) * 128],
                                    wv_t[:, j, ko, :],
                                    start=(j == 0 and ko == 0),
                                    stop=(j == 3 and ko == 1),
                                )
                    for t4 in range(4):
                        nc.vector.tensor_tensor(
                            out=v_sb[:, tg * 4 + t4, :, 0:HD],
                            in0=psv[:, t4, :].rearrange("p (b d) -> p b d", b=2),
                            in1=vb_b[:].rearrange("p (b d) -> p b d", b=2),
                            op=ALU.add,
                        )

                # ---------- causal attention (my 2 heads, all seqs) ----------
                o_loc = dram.tile([O_SZ], BF, tag="o_loc", name="o_loc")
                for s in range(SEQ):
                    o_st = opool.tile([128, 8, 2, HD], BF, tag="ost", name=f"ost{s}")
                    for hp in range(2):
                        p0 = hp * HD
                        po = None
                        recip = None
                        for qt in range(8):
                            # q-tile (s, qt) lives at gathered cols qt*512 + s*128 + j
                            qsl = qT[qt][p0:p0 + HD, s * 128:(s + 1) * 128]
                            nk = qt + 1
                            pt = ptpool.tile([128, 8, 128], BF, tag="pt", name="pt")
                            for half in range((nk + 3) // 4):
                                k0 = half * 4
                                k1 = min(nk, k0 + 4)
                                st = ps_st.tile([128, 4, 128], F32, tag="st", name="st")
                                for kt in range(k0, k1):
                                    nc.tensor.matmul(
                                        st[:, kt - k0, :],
                                        kT[p0:p0 + HD, kt, s * 128:(s + 1) * 128],
                                        qsl,
                                        start=True, stop=True,
                                    )
                                nc.scalar.activation(
                                    pt[:, k0:k1, :], st[:, 0:k1 - k0, :],
                                    AF.Exp, bias=0.0, scale=1.0 / 4096.0,
                                )
                            # causal mask on the diagonal tile only
                            nc.vector.tensor_mul(
                                out=pt[:, qt, :], in0=pt[:, qt, :], in1=tril_t[:])
                            if qt % 4 == 0:
                                po = ps_pv.tile([128, 4, HD + 1], F32, tag="pv", name="po")
                            for kt in range(nk):
                                nc.tensor.matmul(
                                    po[:, qt % 4, :],
                                    pt[:, kt, :],
                                    v_sb[:, kt * 4 + s, hp, :],
                                    start=(kt == 0), stop=(kt == nk - 1),
                                )
                            if qt % 4 == 3:
                                recip = misc.tile([128, 4, 1], F32, tag="recip", name="recip")
                                nc.vector.reciprocal(recip[:], po[:, :, HD:HD + 1])
                                for q2 in range(qt - 3, qt + 1):
                                    nc.vector.tensor_scalar(
                                        out=o_st[:, q2, hp, :], in0=po[:, q2 % 4, 0:HD],
                                        scalar1=recip[:, q2 % 4, :], scalar2=None,
                                        op0=ALU.mult,
                                    )
                    # rows of o_loc block d: (s,j); cols (hp, dd)
                    nc.sync.dma_start(
                        out=_ap(o_loc, s * 128 * 128,
                                [[128, 128], [TL * 128, 8], [HD, 2], [1, HD]]),
                        in_=o_st[:],
                    )

                # prefetch wo / w1 / w2 while attention finishes (1MB chunks so
                # the exclusive DMA device is never held long)
                wo_t = wbig_pool.tile([128, 4, 2, C], BF, tag="w2", name="wo_t")
                nc.sync.dma_start(out=wo_t[:], in_=_w_ap(wo_d, lw * 4 * 256 * C, 4, C))

                o_x = dram.tile([O_SZ], BF, tag="o_x", name="o_x")
                if sim:
                    nc.sync.dma_start(
                        out=_ap(o_x, 0, [[2048, O_SZ // 2048], [1, 2048]]),
                        in_=_ap(o_loc, 0, [[2048, O_SZ // 2048], [1, 2048]]),
                    )
                else:
                    nc.gpsimd.collective_compute(
                        "AllToAll",
                        ALU.bypass,
                        replica_groups=[list(range(NCORES))],
                        ins=[_ap(o_loc, 0, [[TL * 128, NCORES], [2048, 32], [1, 2048]])],
                        outs=[_ap(o_x, 0, [[TL * 128, NCORES], [2048, 32], [1, 2048]])],
                    )

                # read back my tokens' attention output, transpose to c-pairs
                OT = []
                o_rd = vpool.tile([128, SEQ, 8, 2, HD], BF, tag="v", name="ord")
                for s in range(SEQ):
                    nc.sync.dma_start(
                        out=o_rd[:, s, :, :, :],
                        in_=_ap(o_x, s * 128 * 128,
                                [[128, 128], [TL * 128, 8], [HD, 2], [1, HD]]),
                    )
                for j in range(4):
                    pst = ps_st.tile([128, 2, 512], BF, tag="st", name="pst_o")
                    for ko in range(2):
                        cb = 2 * j + ko
                        for s in range(SEQ):
                            nc.tensor.transpose(
                                pst[:, ko, s * 128:(s + 1) * 128],
                                o_rd[:, s, cb, :, :].rearrange("p a b -> p (a b)"),
                                ident[:],
                            )
                    ot = otpool.tile([128, 2, 512], BF, tag=f"OT{j}", name=f"OT{j}")
                    if j % 2 == 0:
                        nc.scalar.activation(ot[:], pst[:], AF.Copy)
                    else:
                        nc.vector.tensor_copy(out=ot[:], in_=pst[:])
                    OT.append(ot)

                # ---------- Wo + residual ----------
                bo_b = bpool.tile([128, C], F32, tag="bb", name="bo_b")
                bcast_row(bo_b, bo_d, lw * C, C)
                for tt in range(SEQ):
                    for nf in range(2):
                        ps = ps_acc.tile([128, 512], F32, tag="acc", name="pso")
                        for j in range(4):
                            for ko in range(2):
                                nc.tensor.matmul(
                                    ps[:], OT[j][:, ko, tt * 128:(tt + 1) * 128],
                                    wo_t[:, j, ko, nf * 512:(nf + 1) * 512],
                                    start=(j == 0 and ko == 0),
                                    stop=(j == 3 and ko == 1),
                                )
                        xs = x_t[tt][:, nf * 512:(nf + 1) * 512]
                        nc.vector.scalar_tensor_tensor(
                            out=xs, in0=ps[:], scalar=1.0 / 64.0, in1=xs,
                            op0=ALU.mult, op1=ALU.add)
                    nc.gpsimd.tensor_tensor(
                        out=x_t[tt][:], in0=x_t[tt][:], in1=bo_b[:], op=ALU.add)

                # ---------- LN2 + FFN (bf16, W1/W2 streamed in halves) ----------
                h2T = emit_ln("hT")  # reuse tags; local tokens only
                b1g_t = misc.tile([128, 32], F32, tag="b1g", name="b1g_t")
                nc.sync.dma_start(out=b1g_t[:], in_=b1g_d[lw])
                ug = ugpool.tile([128, 32, 512], BF, tag="ug", name="ug")
                for fh in range(2):
                    w1_t = wbig_pool.tile([128, 4, 2, FF // 2], BF, tag="w1", name="w1_t")
                    for j in range(4):
                        nc.sync.dma_start(
                            out=w1_t[:, j, :, :],
                            in_=_ap(w1_d, lw * 4 * 256 * FF + j * 128 * 2 * FF + fh * (FF // 2),
                                    [[2 * FF, 128], [FF, 2], [1, FF // 2]]))
                    for fg in range(4):
                        pss = [ps_acc.tile([128, 512], F32, tag="acc", name="psf")
                               for _ in range(4)]
                        for f4 in range(4):
                            fb = fg * 4 + f4
                            for j in range(4):
                                for ko in range(2):
                                    nc.tensor.matmul(
                                        pss[f4][:],
                                        w1_t[:, j, ko, fb * 128:(fb + 1) * 128],
                                        h2T[j][:, ko, :],
                                        start=(j == 0 and ko == 0),
                                        stop=(j == 3 and ko == 1),
                                    )
                        for f4 in range(4):
                            fb = fh * 16 + fg * 4 + f4
                            nc.scalar.activation(
                                ug[:, fb, :], pss[f4][:], AF.Gelu,
                                bias=b1g_t[:, fb:fb + 1], scale=1.0 / 64.0,
                            )
                b2_b = bpool.tile([128, C], F32, tag="bb", name="b2_b")
                bcast_row(b2_b, b2_d, lw * C, C)
                for nf in range(2):
                    w2_t = wbig_pool.tile([128, 16, 2, 512], BF, tag="w2", name="w2_t")
                    for mg in range(4):
                        for mi in range(4):
                            mm = mg * 4 + mi
                            nc.sync.dma_start(
                                out=w2_t[:, mm, :, :],
                                in_=_ap(w2_d, lw * 16 * 256 * C + mm * 256 * C + nf * 512,
                                        [[2 * C, 128], [C, 2], [1, 512]]))
                    for tt in range(SEQ):
                        ps = ps_acc.tile([128, 512], F32, tag="acc", name="ps2")
                        for m in range(16):
                            for ko in range(2):
                                nc.tensor.matmul(
                                    ps[:], ug[:, 2 * m + ko, tt * 128:(tt + 1) * 128],
                                    w2_t[:, m, ko, :],
                                    start=(m == 0 and ko == 0),
                                    stop=(m == 15 and ko == 1),
                                )
                        xs = x_t[tt][:, nf * 512:(nf + 1) * 512]
                        nc.vector.scalar_tensor_tensor(
                            out=xs, in0=ps[:], scalar=1.0 / 64.0, in1=xs,
                            op0=ALU.mult, op1=ALU.add)
                for tt in range(SEQ):
                    nc.vector.tensor_tensor(
                        out=x_t[tt][:], in0=x_t[tt][:], in1=b2_b[:], op=ALU.add)

                if debug:
                    for tt in range(SEQ):
                        nc.sync.dma_start(
                            out=dbg_d[l, tt * 128:(tt + 1) * 128, :], in_=x_t[tt][:]
                        )

            # ---------- final LN, AllGather hf^T, lm_head ----------

            hfT = emit_ln("hT")
            hfT_loc = dram.tile([HT_SZ], BF, tag="hf_loc", name="hf_loc")
            for j in range(4):
                nc.sync.dma_start(
                    out=_ap(hfT_loc, 2 * j * 128 * TL,
                            [[TL, 128], [128 * TL, 2], [1, TL]]),
                    in_=hfT[j][:],
                )
            hfT_full = dram.tile([NCORES * HT_SZ], BF,
                                 addr_space="Local" if sim else "Shared",
                                 tag="hff", name="hff")
            if sim:
                nc.sync.dma_start(
                    out=_ap(hfT_full, 0, [[2048, HT_SZ // 2048], [1, 2048]]),
                    in_=_ap(hfT_loc, 0, [[2048, HT_SZ // 2048], [1, 2048]]),
                )
            else:
                nc.gpsimd.collective_compute(
                    "AllGather",
                    ALU.bypass,
                    replica_groups=[list(range(NCORES))],
                    ins=[_ap(hfT_loc, 0, [[2048, HT_SZ // 2048], [1, 2048]])],
                    outs=[_ap(hfT_full, 0, [[2048, NCORES * HT_SZ // 2048], [1, 2048]])],
                )

            VH = VSH // 2  # 2000
            for vh in range(2):
                wlm_t = wbig_pool.tile([128, 4, 2, VH], BF, tag="w1", name="wlm_t")
                for j in range(4):
                    nc.sync.dma_start(
                        out=wlm_t[:, j, :, :],
                        in_=_ap(wlm_d, j * 128 * 2 * VSH + vh * VH,
                                [[2 * VSH, 128], [VSH, 2], [1, VH]]))
                for r in range(NCORES):
                    hfr = hfr_pool.tile([128, 4, 2, 512], BF, tag="hfr", name="hfr")
                    nc.sync.dma_start(
                        out=hfr[:],
                        in_=_ap(hfT_full, r * HT_SZ,
                                [[TL, 128], [2 * 128 * TL, 4], [128 * TL, 2], [1, TL]]),
                    )
                    for ts in range(SEQ):
                        for half in range(2):
                            lg = lgpool.tile([128, VH // 2], BF,
                                             tag=f"lg{half}", name=f"lg{half}")
                            for v2 in range(2):
                                vc = half * 2 + v2
                                ps = ps_acc.tile([128, VCW], F32, tag="acc", name="psl")
                                for j in range(4):
                                    for ko in range(2):
                                        nc.tensor.matmul(
                                            ps[:],
                                            hfr[:, j, ko, ts * 128:(ts + 1) * 128],
                                            wlm_t[:, j, ko, vc * VCW:(vc + 1) * VCW],
                                            start=(j == 0 and ko == 0),
                                            stop=(j == 3 and ko == 1),
                                        )
                                if vc % 2 == 0:
                                    nc.scalar.activation(
                                        lg[:, v2 * VCW:(v2 + 1) * VCW], ps[:], AF.Copy)
                                else:
                                    nc.vector.tensor_copy(
                                        out=lg[:, v2 * VCW:(v2 + 1) * VCW], in_=ps[:])
                            nc.sync.dma_start(
                                out=logits_d[r, ts, :,
                                             vh * VH + half * (VH // 2):
                                             vh * VH + (half + 1) * (VH // 2)],
                                in_=lg[:])

    nc.compile()
    _prog_cache[key] = nc
    return nc


def _q8(a, scale):
    return np.ascontiguousarray(np.asarray(a, np.float32) * scale).astype(BF16NP)


def _pair_w(w, scale, g=None):
    """[K, D] -> [K/256, 128, 2, D] fp8 with k = (2j+ko)*128+ki, opt. row gain."""
    wf = np.asarray(w, np.float32)
    if g is not None:
        wf = wf * np.asarray(g, np.float32)[:, None]
    k, d = wf.shape
    return _q8(wf.reshape(k // 256, 2, 128, d).transpose(0, 2, 1, 3), scale)


def _prep_inputs(inputs):
    f = {k: np.asarray(k_v) for k, k_v in inputs.items()}
    idx = f["idx"].astype(np.int64)
    emb = f["emb"].astype(np.float32)
    pos = f["pos_enc"].astype(np.float32)
    x_full = emb[idx] + pos[None, :, :]          # [B, T, C] f32

    Wq, Wk, Wv, Wo = (f[k].astype(np.float32) for k in ("Wq", "Wk", "Wv", "Wo"))
    W1, W2 = f["W1"].astype(np.float32), f["W2"].astype(np.float32)
    g1, b1n = f["ln1_g"].astype(np.float32), f["ln1_b"].astype(np.float32)
    g2, b2n = f["ln2_g"].astype(np.float32), f["ln2_b"].astype(np.float32)
    gf, bfn = f["lnf_g"].astype(np.float32), f["lnf_b"].astype(np.float32)
    b1 = f["b1"].astype(np.float32)
    bo, b2v = f["bo"].astype(np.float32), f["b2"].astype(np.float32)
    Wlm = f["Wlm"].astype(np.float32)
    blm = f["blm"].astype(np.float32)

    # layer-folded full tensors
    wq_l, wk_l, wv_l = [], [], []
    qb_l, kb_l, vb_l = [], [], []
    wo_l, w1_l, w2_l, b1g_l = [], [], [], []
    for l in range(L):
        wq_l.append(_pair_w(Wq[l], QS, g1[l]))          # [4,128,2,C]
        wk_l.append(_pair_w(Wk[l], WS, g1[l]))
        wv_l.append(_pair_w(Wv[l], WS, g1[l]))
        qb_l.append(QS * (b1n[l] @ Wq[l]))
        kb_l.append(WS * (b1n[l] @ Wk[l]))
        vb_l.append(WS * (b1n[l] @ Wv[l]))
        wo_l.append(_pair_w(Wo[l], OS))
        w1_l.append(_pair_w(W1[l], WS, g2[l]))
        b1p = b1[l] + b2n[l] @ W1[l]
        b1g_l.append(np.ascontiguousarray(b1p.reshape(32, 128).T, np.float32))
        w2_l.append(_pair_w(W2[l], WS))
    wlm_full = _pair_w(Wlm, WS, gf)                      # [4,128,2,V]
    blm_adj = blm + bfn @ Wlm

    tril = np.ascontiguousarray(
        (np.arange(128)[None, :] >= np.arange(128)[:, None]).astype(np.float32)
    ).astype(BF16NP)

    shared = {
        "tril": tril,
        "bo": bo, "b2": b2v,
        "b1g": np.stack(b1g_l),
        "w1": np.stack(w1_l), "w2": np.stack(w2_l),
        "wo": np.stack(wo_l),
    }

    in_maps = []
    for c in range(NCORES):
        im = dict(shared)
        cs = slice(c * 128, (c + 1) * 128)
        im["x0"] = np.ascontiguousarray(
            x_full[:, cs, :].reshape(TL, C), dtype=np.float32)
        im["wq"] = np.stack([w[:, :, :, cs] for w in wq_l])
        im["wk"] = np.stack([w[:, :, :, cs] for w in wk_l])
        im["wv"] = np.stack([w[:, :, :, cs] for w in wv_l])
        im["qb"] = np.stack([q[cs] for q in qb_l])
        im["kb"] = np.stack([q[cs] for q in kb_l])
        im["vb"] = np.stack([q[cs] for q in vb_l])
        im["wlm"] = np.ascontiguousarray(wlm_full[:, :, :, c * VSH:(c + 1) * VSH])
        in_maps.append(im)
    return in_maps, blm_adj


def kernel(**inputs):
    nc = _build()
    in_maps, blm_adj = _prep_inputs(inputs)
    res = run_bass_kernel_spmd(nc, in_maps, list(range(NCORES)))
    full = np.empty((B, T, V), np.float32)
    for c in range(NCORES):
        part = np.asarray(res.results[c]["logits"], dtype=np.float32)  # [8,4,128,VSH]
        full[:, :, c * VSH:(c + 1) * VSH] = (
            part.transpose(1, 0, 2, 3).reshape(B, T, VSH))
    full = full * (1.0 / 64.0) + blm_adj[None, None, :]
    return np.ascontiguousarray(full, dtype=np.float32)
